# revision 26
# baseline (speedup 1.0000x reference)
# Trainium2 Bass kernel for nn_Decoder_14568529068506 (gnn_message_passing).
#
# Reference computation (per scene s of 32, P=48 peds):
#   rel[i,j]  = obs[j] - obs[i]                  (P,P,2T)   2T=16
#   emb       = rel @ W_se.T                     (P,P,512)
#   emb      *= tile(traj_weight[s])             (P,P,512)
#   x         = concat([emb, h[j]], -1)          (P,P,576)
#   x1        = relu(x @ W1.T + b1)              (P,P,512)
#   x2        = relu(x1 @ W2.T + b2)             (P,P,1024)
#   out[s,i]  = max_j x2[i,j]                    (P,1024)
#
# Kernel restructuring (validated exactly in fp32 numpy):
#  * The tiled traj_weight multiply + spatial embedding + W1 are fused:
#      out1[d,row] = sum_{(ct,g)} Wf[d,(ct,g)] * tw[row,ct] * rel[row,g]
#    with Wf[d, ct*16+g] = sum_{k%2==c} W1[d, t*64+k] * W_se[t*64+k, g].
#    So MLP1 contracts over 256 "rel2" features (+64 h features) instead
#    of 576, and the (P,P,512) embedding is never materialized.
#  * rel2 = tw_rep * rel_rep feature-major on 128 partitions:
#      rel_rep = obs_rep.T @ D   (D = +-1 pairwise difference matrix, PE)
#      tw_rep  = 16x partition-replicated tw, built by a broadcast DMA
#      (reads each tw row 16x from DRAM; large 2.3KB packets)
#    so prep costs one matmul + two vector multiplies per row block.
#  * The h-state part of MLP1 rides as a third K=64 accumulation matmul
#    whose rhs (h broadcast over i) is built once per scene.
#  * relu/bias commute with max-pool: MLP2 PSUM outputs are max-pooled
#    directly; bias+relu run post-pool on the scalar engine.
#  * Output is written TRANSPOSED ([1024, P] per scene) straight from the
#    pooled tile via DMA; the host transposes back. This removes the PE
#    transposes and vector staging copies of the previous version.
#  * Matmuls run in bf16 (1 col/cycle @2.4GHz; LDWEIGHTS ~97ns hides under
#    the 160ns N=384 stream). PSUM accumulation stays fp32.
#  * PSUM: 3 single-tag pools (rel 2 banks, mlp1 3, mlp2 3) so slot
#    recycling never serializes PE on a vector-engine consumer.
#
# Sharding: scenes are data-parallel across the 8 cores (4 scenes each);
# weights replicated; per-core outputs concatenated on the host.

import numpy as np

S, P, T, E, H = 32, 48, 8, 64, 64
D1, D2 = 512, 1024
B = S * P
NCORES = 8
SC = S // NCORES          # scenes per core
NB = 6                    # row blocks per scene
NBLK = P * P // NB        # 384 columns (pairs) per block = 8 i-groups x 48 j
IB = NBLK // P            # i-groups per block (8)


def _host_constants(W_se, W1, W2, b1, b2):
    """Precompute fused weights + structural constant matrices (fp32)."""
    W_se = np.asarray(W_se, np.float32)
    W1 = np.asarray(W1, np.float32)
    W2 = np.asarray(W2, np.float32)
    b1 = np.asarray(b1, np.float32)
    b2 = np.asarray(b2, np.float32)

    W1e, W1h = W1[:, :512], W1[:, 512:]
    Wf = np.zeros((D1, 256), np.float32)
    for c in range(2):
        for t in range(T):
            ct = c * 8 + t
            f = t * 64 + np.arange(c, 64, 2)
            Wf[:, ct * 16:(ct + 1) * 16] = W1e[:, f] @ W_se[f, :]

    # Dm zero-padded to K=128: all matmuls use the full 128-row PE group
    # (switching PE row groups costs ~100ns each way on TRN2)
    Dm = np.zeros((128, P * P), np.float32)
    ii, jj = np.meshgrid(np.arange(P), np.arange(P), indexing="ij")
    rows = (ii * P + jj).ravel()
    np.add.at(Dm, (jj.ravel(), rows), 1.0)
    np.add.at(Dm, (ii.ravel(), rows), -1.0)

    # lhsT tile layouts: [128, kTiles, M] so DMAs are contiguous
    Wf_sb = np.ascontiguousarray(Wf.T.reshape(2, 128, D1).transpose(1, 0, 2))
    # W1h/2 stacked twice (rows exact in bf16): h rides K=128 with h
    # replicated to both partition halves
    W1h_sb = np.ascontiguousarray(np.vstack([W1h.T, W1h.T]) * 0.5)  # (128, 512)
    W2_sb = np.ascontiguousarray(W2.T.reshape(4, 128, D2).transpose(1, 0, 2))
    b1_sb = np.ascontiguousarray(b1.reshape(4, 128).T)       # (128, 4)
    b2_sb = np.ascontiguousarray(b2.reshape(8, 128).T)       # (128, 8)
    return dict(Wf_sb=Wf_sb, W1h_sb=W1h_sb, W2_sb=W2_sb, b1_sb=b1_sb,
                b2_sb=b2_sb, Dm=Dm)


def build_program(n_scenes=SC):
    """Emit the per-core Bass/Tile program. Returns the compiled Bacc."""
    from contextlib import ExitStack
    import concourse.bacc as bacc
    import concourse.tile as tile
    from concourse import mybir
    from concourse.alu_op_type import AluOpType

    f32 = mybir.dt.float32
    bf16 = mybir.dt.bfloat16
    AF = mybir.ActivationFunctionType
    AX = mybir.AxisListType

    nc = bacc.Bacc("TRN2", target_bir_lowering=False, debug=False)

    # ---- DRAM parameters -------------------------------------------------
    d_obs = nc.dram_tensor("obs_rm", [n_scenes * P, 16], bf16, kind="ExternalInput")
    d_tw0 = nc.dram_tensor("tw0r", [n_scenes, 128, P * P], bf16, kind="ExternalInput")
    d_tw1 = nc.dram_tensor("tw1r", [n_scenes, 128, P * P], bf16, kind="ExternalInput")
    d_h = nc.dram_tensor("h_fm", [n_scenes, 64, P], bf16, kind="ExternalInput")
    d_Dm = nc.dram_tensor("Dm", [P, P * P], bf16, kind="ExternalInput")
    d_Wf = nc.dram_tensor("Wf_sb", [128, 2, D1], bf16, kind="ExternalInput")
    d_W1h = nc.dram_tensor("W1h_sb", [128, D1], bf16, kind="ExternalInput")
    d_W2 = nc.dram_tensor("W2_sb", [128, 4, D2], bf16, kind="ExternalInput")
    d_b1 = nc.dram_tensor("b1_sb", [128, 4], f32, kind="ExternalInput")
    d_b2 = nc.dram_tensor("b2_sb", [128, 8], f32, kind="ExternalInput")
    # transposed output: out[d2, scene*P + i]; host transposes back
    d_out = nc.dram_tensor("out", [D2, n_scenes * P], f32, kind="ExternalOutput")

    with ExitStack() as ctx:
        tc = ctx.enter_context(tile.TileContext(nc))
        consts = ctx.enter_context(tc.tile_pool(name="consts", bufs=1))
        scene_pool = ctx.enter_context(tc.tile_pool(name="scene", bufs=2))
        blk_pool = ctx.enter_context(tc.tile_pool(name="blk", bufs=3))
        # PSUM: 1 + 3 + 2x2 banks. p2 tiles span two banks so one reduce
        # covers two MLP2 m-tiles (halves reduce count and PE stop-semaphore
        # updates, which cost ~100ns each on the PE pipeline).
        pp = ctx.enter_context(tc.tile_pool(name="pp", bufs=1, space="PSUM"))
        p1 = ctx.enter_context(tc.tile_pool(name="p1", bufs=3, space="PSUM"))
        p2 = ctx.enter_context(tc.tile_pool(name="p2", bufs=2, space="PSUM"))

        Dm_sb = consts.tile([128, P * P], bf16)
        b1_sb = consts.tile([128, 4], f32)
        b2_sb = consts.tile([128, 8], f32)
        Wf_sb = consts.tile([128, 2, D1], bf16)
        W1h_sb = consts.tile([128, D1], bf16)
        W2_sb = consts.tile([128, 4, D2], bf16)

        blocks = [(s, b) for s in range(n_scenes) for b in range(NB)]
        state = {}   # per-scene tiles

        def scene_setup(s, split=False):
            # tw arrives pre-replicated 16x along partitions from the host;
            # two plain contiguous [128, 4.6KB] loads per scene. For scene 0
            # only the first-half columns load now; the rest queues behind
            # the weights (blocks 0-2 touch cols < P*P/2 only).
            tw0 = scene_pool.tile([128, P * P], bf16, tag="tw0")
            tw1 = scene_pool.tile([128, P * P], bf16, tag="tw1")
            if not split:     # scene 0's tw loads are sequenced by the caller
                nc.sync.dma_start(tw0[:], d_tw0[s])
                nc.sync.dma_start(tw1[:], d_tw1[s])
            obs_c = scene_pool.tile([P, 16], bf16, tag="obs_c")
            nc.sync.dma_start(obs_c[:], d_obs[s * P:(s + 1) * P, :])
            # obs_rep padded to K=128; rows P.. are zeroed (Dm rows are 0
            # there, but uninitialized SBUF could hold Inf/NaN patterns)
            obs_rep = scene_pool.tile([128, 128], bf16, tag="obs_rep")
            nc.vector.memset(obs_rep[:], 0.0)
            nc.vector.tensor_copy(
                obs_rep[:P].rearrange("p (r g) -> p r g", r=8),
                obs_c[:].unsqueeze(1).broadcast_to([P, 8, 16]))
            # h broadcast over i: hj_fm[:, ii*P + j] = h_fm[s, :, j];
            # replicated into both partition halves (W1h_sb holds W1h/2 twice)
            h_c = scene_pool.tile([128, P], bf16, tag="h_c")
            nc.sync.dma_start(h_c[:64], d_h[s])
            nc.sync.dma_start(h_c[64:], d_h[s])
            hj_fm = scene_pool.tile([128, NBLK], bf16, tag="hj_fm")
            nc.vector.tensor_copy(
                hj_fm[:].rearrange("p (r j) -> p r j", r=IB),
                h_c[:].unsqueeze(1).broadcast_to([128, IB, P]))
            pooled = scene_pool.tile([128, 8, P], f32, tag="pooled")
            state[s] = dict(tw0=tw0, tw1=tw1, obs_rep=obs_rep, hj_fm=hj_fm,
                            pooled=pooled)
            return tw0, tw1

        def prep(s, b):
            st = state[s]
            c0 = b * NBLK
            rel_ps = pp.tile([128, NBLK], f32, tag="pp")
            nc.tensor.matmul(rel_ps[:], st["obs_rep"][:],
                             Dm_sb[:, c0:c0 + NBLK], start=True, stop=True,
                             tile_position=(0, 0))
            rel2_0 = blk_pool.tile([128, NBLK], bf16, tag="rel2_0")
            nc.vector.tensor_tensor(rel2_0[:], st["tw0"][:, c0:c0 + NBLK],
                                    rel_ps[:], AluOpType.mult)
            rel2_1 = blk_pool.tile([128, NBLK], bf16, tag="rel2_1")
            nc.vector.tensor_tensor(rel2_1[:], st["tw1"][:, c0:c0 + NBLK],
                                    rel_ps[:], AluOpType.mult)
            return dict(rel2_0=rel2_0, rel2_1=rel2_1, s=s, b=b)

        def mlp1(job):
            st = state[job["s"]]
            r20 = job["rel2_0"][:]
            r21 = job["rel2_1"][:]
            x1 = blk_pool.tile([128, 4, NBLK], bf16, tag="x1")
            for m in range(4):
                p1t = p1.tile([128, NBLK], f32, tag="p1")
                nc.tensor.matmul(p1t[:], Wf_sb[:, 0, m * 128:(m + 1) * 128],
                                 r20, start=True, stop=False)
                nc.tensor.matmul(p1t[:], Wf_sb[:, 1, m * 128:(m + 1) * 128],
                                 r21, start=False, stop=False)
                nc.tensor.matmul(p1t[:], W1h_sb[:, m * 128:(m + 1) * 128],
                                 st["hj_fm"][:], start=False, stop=True)
                nc.scalar.activation(x1[:, m, :], p1t[:], AF.Relu,
                                     bias=b1_sb[:, m:m + 1])
            job["x1"] = x1

        def mlp2_mpair(job, mp, p2t=None):
            """MLP2 m-tiles 2*mp, 2*mp+1 of one block into a 2-bank PSUM
            pair tile, then one reduce covering both. Returns the pair tile
            for reuse by a sibling block."""
            s, b = job["s"], job["b"]
            x1 = job["x1"]
            if p2t is None:
                p2t = p2.tile([128, 2, 512], f32, tag="p2")
            for half in range(2):
                mm = 2 * mp + half
                for k in range(4):
                    nc.tensor.matmul(
                        p2t[:, half, :NBLK], W2_sb[:, k, mm * 128:(mm + 1) * 128],
                        x1[:, k, :], start=(k == 0), stop=(k == 3))
            nc.vector.tensor_reduce(
                state[s]["pooled"][:, 2 * mp:2 * mp + 2, b * IB:(b + 1) * IB],
                p2t[:, :, :NBLK].rearrange("p h (i j) -> p h i j", j=P),
                axis=AX.X, op=AluOpType.max)
            return p2t

        def finish_m(s, mm):
            # scene output for m-tile mm: bias+relu post-pool on the scalar
            # engine, then DMA the [128, P] slice (output is transposed).
            # DMAs alternate issue rings: each is 128 small (192B) packets,
            # and a single ring drains them too slowly at the kernel tail.
            pooled = state[s]["pooled"]
            nc.scalar.activation(pooled[:, mm, :], pooled[:, mm, :],
                                 AF.Relu, bias=b2_sb[:, mm:mm + 1])
            eng = nc.sync if mm % 2 == 0 else nc.scalar
            eng.dma_start(
                d_out[mm * 128:(mm + 1) * 128, s * P:(s + 1) * P],
                pooled[:, mm, :])

        def mlp2(job):
            s, b = job["s"], job["b"]
            last = b == NB - 1
            for mp in range(4):
                mlp2_mpair(job, mp)
                if last:
                    finish_m(s, 2 * mp)
                    finish_m(s, 2 * mp + 1)
            if last:
                state.pop(s)

        def mlp2_final_pair(job_a, job_b):
            """Last two blocks of the final scene, m-pair interleaved so the
            vector-engine reduce queue keeps pace with PE and the kernel
            tail is one m-pair deep instead of two blocks deep."""
            s = job_b["s"]
            for mp in range(4):
                mlp2_mpair(job_a, mp)
                mlp2_mpair(job_b, mp)
                finish_m(s, 2 * mp)
                finish_m(s, 2 * mp + 1)
            state.pop(s)

        # two-deep software pipeline on PE:
        #   ... prep(i)  mlp1(i-1)  mlp2(i-2) ...
        # so x1 is ready a full block before MLP2 consumes it; scene data is
        # prefetched two blocks before the scene starts; weights stream in
        # behind scene 0's data.
        # Startup loads split across the three DMA-issuing engines so issue
        # serialization (~620ns per dma_start) doesn't stretch the warmup:
        #   sync:   scene-0 data (obs, h, tw halves)
        #   scalar: Dm rows + Wf + W2 (PE critical path)
        #   gpsimd: W1h, biases
        # Dm rows 48.. are zeroed once by a full-tile memset underneath.
        nc.vector.memset(Dm_sb[:], 0.0)
        tw0_0, tw1_0 = scene_setup(0, split=True)
        HPP = P * P // 2
        nc.scalar.dma_start(Dm_sb[:P, :], d_Dm[:])
        nc.scalar.dma_start(Wf_sb[:, 0], d_Wf[:, 0])
        nc.scalar.dma_start(Wf_sb[:, 1], d_Wf[:, 1])
        nc.sync.dma_start(tw0_0[:, :HPP], d_tw0[0, :, :HPP])
        nc.sync.dma_start(tw1_0[:, :HPP], d_tw1[0, :, :HPP])
        nc.gpsimd.dma_start(W1h_sb[:], d_W1h[:])
        nc.gpsimd.dma_start(b1_sb[:], d_b1[:])
        # W2 split across all three rings so the last chunk lands early
        nc.scalar.dma_start(W2_sb[:, 0], d_W2[:, 0])
        nc.scalar.dma_start(W2_sb[:, 1], d_W2[:, 1])
        nc.sync.dma_start(W2_sb[:, 2], d_W2[:, 2])
        nc.gpsimd.dma_start(W2_sb[:, 3], d_W2[:, 3])
        nc.gpsimd.dma_start(b2_sb[:], d_b2[:])
        nc.sync.dma_start(tw0_0[:, HPP:], d_tw0[0, :, HPP:])
        nc.sync.dma_start(tw1_0[:, HPP:], d_tw1[0, :, HPP:])

        # emission order per slot i: mlp2(i-2), prep(i), mlp1(i-1) — reduces
        # of block i-2 queue on the vector engine BEFORE block i's rel2
        # multiplies, so in the drain mlp1 of the final block isn't stuck
        # behind a reduce backlog. The final block's prep is emitted one
        # slot early for the same reason.
        NBL = len(blocks)
        jobs = {}
        for i, (s, b) in enumerate(blocks):
            if i >= 2:
                mlp2(jobs.pop(i - 2))
            if b == NB - 2 and s + 1 < n_scenes:
                scene_setup(s + 1)
            if i < NBL - 1:
                jobs[i] = prep(s, b)
                if i == NBL - 2:
                    jobs[NBL - 1] = prep(*blocks[NBL - 1])
            if i >= 1:
                mlp1(jobs[i - 1])
        mlp1(jobs[NBL - 1])
        mlp2_final_pair(jobs.pop(NBL - 2), jobs.pop(NBL - 1))

    nc.compile()
    return nc


def _host_inputs(h_states, traj, traj_weight, consts, n_scenes=SC):
    """Slice + lay out per-core input maps (matmul operands cast to bf16)."""
    import ml_dtypes
    bf = ml_dtypes.bfloat16
    h_states = np.asarray(h_states, np.float32)
    traj = np.asarray(traj, np.float32)
    traj_weight = np.asarray(traj_weight, np.float32)

    obs_full = np.ascontiguousarray(
        traj[:T].transpose(1, 0, 2).reshape(B, 2 * T))          # (B,16) g=t*2+c
    h_full = h_states.reshape(S, P, H)

    consts = dict(consts)
    consts["Dm"] = consts["Dm"][:P]      # rows P.. are zeroed on-device
    for k in ("Wf_sb", "W1h_sb", "W2_sb", "Dm"):
        consts[k] = np.ascontiguousarray(consts[k]).astype(bf)

    in_maps = []
    for core in range(NCORES):
        s0 = core * n_scenes
        sl = slice(s0, s0 + n_scenes)
        twT = np.ascontiguousarray(
            traj_weight[sl].transpose(0, 2, 3, 1).reshape(n_scenes, 16, P * P))
        # pre-replicate each tw row 16x along partitions (feature-major)
        tw0r = np.ascontiguousarray(np.repeat(twT[:, 0:8], 16, axis=1)).astype(bf)
        tw1r = np.ascontiguousarray(np.repeat(twT[:, 8:16], 16, axis=1)).astype(bf)
        h_fm = np.ascontiguousarray(h_full[sl].transpose(0, 2, 1)).astype(bf)
        obs_rm = np.ascontiguousarray(
            obs_full[s0 * P:(s0 + n_scenes) * P]).astype(bf)
        m = dict(obs_rm=obs_rm, tw0r=tw0r, tw1r=tw1r, h_fm=h_fm)
        m.update(consts)
        in_maps.append(m)
    return in_maps


def kernel(h_states, seq_start_end, end_pos, traj, traj_weight,
           mlp_pre_pool_dim_0, W_se, b_se, W1, b1, W2, b2):
    import sys
    if '/opt/trn_rl_repo' not in sys.path:
        sys.path.insert(0, '/opt/trn_rl_repo')
    from concourse.bass_utils import run_bass_kernel_spmd

    consts = _host_constants(W_se, W1, W2, b1, b2)
    in_maps = _host_inputs(h_states, traj, traj_weight, consts)
    nc = build_program(SC)
    res = run_bass_kernel_spmd(nc, in_maps, list(range(NCORES)))
    out = np.concatenate(
        [res.results[i]["out"].T for i in range(NCORES)], axis=0)
    return np.ascontiguousarray(out).astype(np.float32)
